# revision 1
# baseline (speedup 1.0000x reference)
"""Trainium2 Bass kernel: Encoder_HieStackedCorr (UnCorrVmat_Detail, t_method='uncorr').

Math (per batch b):
    W1 = wn(U1_v, U1_g); W2 = wn(U2_v, U2_g)
    R = relu(V @ W1.T + b1)          [N, LR]
    L = relu(V @ W2.T + b2)          [N, LR]
    UnCorr = L @ R.T                 [N, N]
    d[n] = UnCorr[n, n] = sum_l L[n,l] R[n,l]
    dr = 1/sqrt(d + eps)
    A = 1 + I - dr dr^T * UnCorr
    v = mean_n (A @ V) = (1/N) * s @ V  where s[m] = N + 1 - dr[m] * (t . R[m,:]),
                                              t = sum_n dr[n] L[n,:]
    feat = v @ W_lin.T + b_lin ; out = batchnorm(feat)   (training-mode stats)

The N x N matrix is never materialized: mean-pooling commutes with the matmul,
collapsing the O(B N^2 (LR+D)) reference into O(B N D LR) work.

Sharding: data-parallel over batch, 4 batches per core on 8 cores.  Each core
computes v for its 4 batches; the tiny [32,256] linear + batchnorm epilogue
(needs cross-core batch stats) runs on host.
"""

import os
import sys

import numpy as np

for _p in ("/opt/trn_rl_repo", "/root/.axon_site/_ro/trn_rl_repo"):
    if os.path.isdir(_p) and _p not in sys.path:
        sys.path.insert(0, _p)
        break

import ml_dtypes  # noqa: E402
import concourse.bass as bass  # noqa: E402
import concourse.bacc as bacc  # noqa: E402
import concourse.mybir as mybir  # noqa: E402
import concourse.tile as tile  # noqa: E402
from concourse.bass_utils import run_bass_kernel_spmd  # noqa: E402


def _ensure_ntff_hook():
    """Shim the missing ``antenv.axon_hooks`` registry so trace=True works.

    The agent image's ``antenv`` lacks ``axon_hooks``; the ctypes NTFF driver
    in ``trn_agent_boot.trn_boot`` is present and the injected libaxon_pjrt.so
    exports the profile symbols, so wire them together here.
    """
    import types

    try:
        from antenv.axon_hooks import get_axon_ntff_profile_hook  # noqa: F401
        return
    except ImportError:
        pass
    try:
        from trn_agent_boot.trn_boot import _ntff_profile_via_ctypes
        hook = _ntff_profile_via_ctypes("/opt/axon/libaxon_pjrt.so")
    except Exception:
        hook = None
    mod = types.ModuleType("antenv.axon_hooks")
    mod._hook = hook
    mod.get_axon_ntff_profile_hook = lambda: mod._hook
    mod.set_axon_ntff_profile_hook = lambda h: setattr(mod, "_hook", h)
    sys.modules["antenv.axon_hooks"] = mod


_ensure_ntff_hook()

# Problem constants (hardcoded; see module docstring).
B, N, D, LR, EMB = 32, 2048, 256, 64, 256
NCORES = 8
B_LOC = B // NCORES          # 4 batches per core
ROWS = B_LOC * N             # 8192 rows per core
NT_B = N // 128              # 16 row-tiles per batch
NBLK = N // 512              # 4 512-row blocks per batch
EPS_DIAG = 1e-6
EPS_BN = 1e-5

F32 = mybir.dt.float32
BF16 = mybir.dt.bfloat16

# dt: dtype for V/Vt/W, the L/R activations and every matmul operand
# ("f32" = exact but 4 cyc/row on the PE, "bf16" = 1 cyc/row).
CONFIG = dict(dt="f32", trace=False)

_CACHE = {}


def _build(cfg):
    DT = BF16 if cfg["dt"] == "bf16" else F32
    nc = bacc.Bacc("TRN2", target_bir_lowering=False, debug=False)

    v_d = nc.dram_tensor("v", [ROWS, D], DT, kind="ExternalInput").ap()
    vt_d = nc.dram_tensor("vt", [2, 128, ROWS], DT, kind="ExternalInput").ap()
    w1_d = nc.dram_tensor("w1t", [2, 128, LR], DT, kind="ExternalInput").ap()
    w2_d = nc.dram_tensor("w2t", [2, 128, LR], DT, kind="ExternalInput").ap()
    b1_d = nc.dram_tensor("b1", [LR, 1], F32, kind="ExternalInput").ap()
    b2_d = nc.dram_tensor("b2", [LR, 1], F32, kind="ExternalInput").ap()
    out_d = nc.dram_tensor("vmean", [1, B_LOC * D], F32, kind="ExternalOutput").ap()

    with tile.TileContext(nc) as tc:
        with (
            tc.tile_pool(name="const", bufs=1) as cpool,
            tc.tile_pool(name="vst", bufs=1) as vpool,
            tc.tile_pool(name="lrbuf", bufs=2) as lrpool,
            tc.tile_pool(name="blk", bufs=3) as bpool,
            tc.tile_pool(name="rows", bufs=2) as rpool,
            tc.tile_pool(name="ps_lr", bufs=2, space="PSUM") as ps_lr,
            tc.tile_pool(name="ps_d", bufs=1, space="PSUM") as ps_d,
            tc.tile_pool(name="ps_misc", bufs=1, space="PSUM") as ps_misc,
            tc.tile_pool(name="dram", bufs=2, space="DRAM") as dpool,
        ):
            # ---- constants / weights ----
            w1_sb = cpool.tile([128, 2 * LR], DT)
            w2_sb = cpool.tile([128, 2 * LR], DT)
            nc.sync.dma_start(
                w1_sb[:].rearrange("p (c l) -> p c l", c=2),
                w1_d.rearrange("c p l -> p c l"),
            )
            nc.sync.dma_start(
                w2_sb[:].rearrange("p (c l) -> p c l", c=2),
                w2_d.rearrange("c p l -> p c l"),
            )
            b1_sb = cpool.tile([LR, 1], F32)
            b2_sb = cpool.tile([LR, 1], F32)
            nc.sync.dma_start(b1_sb[:], b1_d[:])
            nc.sync.dma_start(b2_sb[:], b2_d[:])
            ones64 = cpool.tile([LR, 1], DT)
            nc.vector.memset(ones64[:], 1.0)
            ones_k1 = cpool.tile([1, LR], DT)
            nc.vector.memset(ones_k1[:], 1.0)
            eps_sb = cpool.tile([1, 1], F32)
            nc.vector.memset(eps_sb[:], EPS_DIAG)

            out_sb = cpool.tile([1, B_LOC * D], F32)

            # per-batch persistent tiles, double buffered across batches
            v_t = {}
            vt_t = {}
            for b in range(B_LOC):
                # natural V for this batch: tile j at cols [j*D, (j+1)*D)
                v_t[b] = vpool.tile([128, NT_B * D], DT, tag="vnat", name=f"vnat{b}")
                # transposed V, both d-chunks: chunk c at cols [c*N, (c+1)*N)
                vt_t[b] = vpool.tile([128, 2 * N], DT, tag="vt", name=f"vt{b}")
                src = v_d.rearrange("(t p) d -> p t d", p=128)
                nc.sync.dma_start(
                    v_t[b][:].rearrange("p (t d) -> p t d", t=NT_B),
                    src[:, b * NT_B:(b + 1) * NT_B, :],
                )
                nc.sync.dma_start(
                    vt_t[b][:].rearrange("p (c n) -> p c n", c=2),
                    vt_d[:, :, b * N:(b + 1) * N].rearrange("c p n -> p c n"),
                )

            for b in range(B_LOC):
                L_sb = lrpool.tile([LR, N], DT, tag="L")
                R_sb = lrpool.tile([LR, N], DT, tag="R")
                sq_row = rpool.tile([1, N], F32, tag="sq")     # sqrt(d + eps)
                dr_row = rpool.tile([1, N], F32, tag="dr")     # 1/sqrt(d + eps)
                s_row = rpool.tile([1, N], F32, tag="s")       # ((N+1) - c)/N
                for blk in range(NBLK):
                    f0 = blk * 512
                    # L/R = V @ W.T in transposed layout [LR, n-block]
                    L_ps = ps_lr.tile([LR, 512], F32, tag="Lps")
                    R_ps = ps_lr.tile([LR, 512], F32, tag="Rps")
                    for c in range(2):
                        rhs = vt_t[b][:, c * N + f0:c * N + f0 + 512]
                        nc.tensor.matmul(
                            L_ps[:], w2_sb[:, c * LR:(c + 1) * LR], rhs,
                            start=(c == 0), stop=(c == 1),
                        )
                        nc.tensor.matmul(
                            R_ps[:], w1_sb[:, c * LR:(c + 1) * LR], rhs,
                            start=(c == 0), stop=(c == 1),
                        )
                    # relu(+bias): R on ACT, L on DVE (balance engines)
                    nc.scalar.activation(
                        R_sb[:, f0:f0 + 512], R_ps[:],
                        mybir.ActivationFunctionType.Relu, bias=b1_sb[:], scale=1.0,
                    )
                    nc.vector.tensor_scalar(
                        L_sb[:, f0:f0 + 512], L_ps[:], b2_sb[:], 0.0,
                        mybir.AluOpType.add, mybir.AluOpType.max,
                    )
                    # diag: d[n] = sum_l L[l,n]*R[l,n] -> [1,512] via ones-matmul
                    prod = bpool.tile([LR, 512], DT, tag="prod")
                    nc.vector.tensor_tensor(
                        prod[:], L_sb[:, f0:f0 + 512], R_sb[:, f0:f0 + 512],
                        mybir.AluOpType.mult,
                    )
                    d_ps = ps_d.tile([1, 512], F32, tag="dps")
                    nc.tensor.matmul(
                        d_ps[:], ones64[:], prod[:],
                        start=True, stop=True,
                    )
                    # dr = 1/sqrt(d + eps)  (Rsqrt is banned on ACT; DVE reciprocal)
                    nc.scalar.activation(
                        sq_row[:, f0:f0 + 512], d_ps[:],
                        mybir.ActivationFunctionType.Sqrt, bias=eps_sb[:], scale=1.0,
                    )
                    nc.vector.reciprocal(
                        dr_row[:, f0:f0 + 512], sq_row[:, f0:f0 + 512]
                    )

                # t = sum_n dr[n] * L[n,:]   (chained fused multiply-reduce)
                dr_dt = dr_row
                if DT != F32:
                    dr_dt = rpool.tile([1, N], DT, tag="dr_dt", name=f"drdt{b}")
                    nc.scalar.activation(
                        dr_dt[:], dr_row[:], mybir.ActivationFunctionType.Copy
                    )
                ldr = lrpool.tile([LR, N], DT, tag="ldr", name=f"ldr{b}")
                for blk in range(NBLK):
                    f0 = blk * 512
                    rep_ps = ps_misc.tile([LR, 512], F32, tag="rep")
                    nc.tensor.matmul(
                        rep_ps[:], ones_k1[:], dr_dt[:, f0:f0 + 512],
                        start=True, stop=True,
                    )
                    nc.vector.tensor_tensor(
                        ldr[:, f0:f0 + 512], L_sb[:, f0:f0 + 512], rep_ps[:],
                        mybir.AluOpType.mult,
                    )
                t_sb = bpool.tile([LR, 1], F32, tag="t", name=f"tacc{b}")
                nc.vector.tensor_reduce(
                    t_sb[:], ldr[:], mybir.AxisListType.X, mybir.AluOpType.add,
                )
                t_dt = t_sb
                if DT != F32:
                    t_dt = bpool.tile([LR, 1], DT, tag="t_dt", name=f"tdt{b}")
                    nc.scalar.activation(
                        t_dt[:], t_sb[:], mybir.ActivationFunctionType.Copy
                    )

                # u = t . R[m,:] -> [1,512] blocks; c = u / sq; s = ((N+1)-c)/N
                for blk in range(NBLK):
                    f0 = blk * 512
                    u_ps = ps_misc.tile([1, 512], F32, tag="ups")
                    nc.tensor.matmul(
                        u_ps[:], t_dt[:], R_sb[:, f0:f0 + 512],
                        start=True, stop=True,
                    )
                    c_row = bpool.tile([1, 512], F32, tag="crow")
                    nc.vector.tensor_tensor(
                        c_row[:], u_ps[:], dr_row[:, f0:f0 + 512],
                        mybir.AluOpType.mult,
                    )
                    nc.scalar.activation(
                        s_row[:, f0:f0 + 512], c_row[:],
                        mybir.ActivationFunctionType.Copy,
                        bias=float(N + 1) / N, scale=-1.0 / N,
                    )

                # scatter s to partitions: s_col[p, j] = s[j*128 + p].
                # A direct SBUF->SBUF rearrange is NOT usable: the source AP's
                # first dim is interpreted as physical partitions by the DMA
                # descriptor generator (HW reads partitions 1.. as garbage).
                # Bounce through DRAM, where APs are plain strided views.
                s_dram = dpool.tile([1, N], F32, tag="sdram", name=f"sdram{b}")
                nc.sync.dma_start(s_dram[:], s_row[:])
                s_col = bpool.tile([128, NT_B], F32, tag="scol")
                nc.sync.dma_start(
                    s_col[:], s_dram.rearrange("a (j p) -> (a p) j", p=128)
                )
                s_dt = s_col
                if DT != F32:
                    s_dt = bpool.tile([128, NT_B], DT, tag="scol_dt")
                    nc.scalar.activation(
                        s_dt[:], s_col[:], mybir.ActivationFunctionType.Copy
                    )

                # v_mean = s^T @ V  (accumulate over the 16 row-tiles)
                v_ps = ps_misc.tile([1, D], F32, tag="vps")
                for j in range(NT_B):
                    nc.tensor.matmul(
                        v_ps[:], s_dt[:, j:j + 1],
                        v_t[b][:, j * D:(j + 1) * D],
                        start=(j == 0), stop=(j == NT_B - 1),
                    )
                nc.scalar.activation(
                    out_sb[:, b * D:(b + 1) * D], v_ps[:],
                    mybir.ActivationFunctionType.Copy,
                )

            nc.sync.dma_start(out_d[:], out_sb[:])

    nc.compile()
    return nc


def _host_prep(inputs, cfg):
    """Weight-norm, transposes, casts; returns per-core input maps + epilogue data."""
    np_dt = ml_dtypes.bfloat16 if cfg["dt"] == "bf16" else np.float32

    def wn(v, g):
        return v * (g / np.linalg.norm(v.astype(np.float64), axis=1)).astype(
            np.float32
        )[:, None]

    W1 = wn(np.asarray(inputs["U1_v"], np.float32), np.asarray(inputs["U1_g"], np.float32))
    W2 = wn(np.asarray(inputs["U2_v"], np.float32), np.asarray(inputs["U2_g"], np.float32))
    w1t = np.ascontiguousarray(W1.T).reshape(2, 128, LR).astype(np_dt)
    w2t = np.ascontiguousarray(W2.T).reshape(2, 128, LR).astype(np_dt)
    b1 = np.asarray(inputs["U1_b"], np.float32).reshape(LR, 1)
    b2 = np.asarray(inputs["U2_b"], np.float32).reshape(LR, 1)

    V = np.asarray(inputs["Vmat"], np.float32)  # [B, N, D]
    in_maps = []
    for k in range(NCORES):
        Vk = np.ascontiguousarray(V[k * B_LOC:(k + 1) * B_LOC].reshape(ROWS, D))
        vt = np.ascontiguousarray(Vk.T).reshape(2, 128, ROWS).astype(np_dt)
        in_maps.append({
            "v": Vk.astype(np_dt),
            "vt": vt,
            "w1t": w1t,
            "w2t": w2t,
            "b1": b1,
            "b2": b2,
        })
    return in_maps


def _epilogue(v_mean, inputs):
    """feat = v_mean @ W_lin.T + b_lin, then training-mode batchnorm."""
    W_lin = np.asarray(inputs["W_lin"], np.float32)
    b_lin = np.asarray(inputs["b_lin"], np.float32)
    gamma = np.asarray(inputs["gamma"], np.float32)
    beta = np.asarray(inputs["beta"], np.float32)
    feat = v_mean.astype(np.float32) @ W_lin.T + b_lin
    mu = feat.mean(axis=0)
    var = feat.var(axis=0)
    out = (feat - mu) / np.sqrt(var + EPS_BN) * gamma + beta
    return out.astype(np.float32)


def kernel(**inputs):
    cfg = dict(CONFIG)
    key = (cfg["dt"],)
    if key not in _CACHE:
        _CACHE[key] = _build(cfg)
    nc = _CACHE[key]
    in_maps = _host_prep(inputs, cfg)
    res = run_bass_kernel_spmd(
        nc, in_maps, core_ids=list(range(NCORES)), trace=cfg["trace"]
    )
    kernel.last_results = res
    v_mean = np.concatenate(
        [res.results[k]["vmean"].reshape(B_LOC, D) for k in range(NCORES)], axis=0
    )
    return _epilogue(v_mean, inputs)



# revision 9
# speedup vs baseline: 1.1139x; 1.1139x over previous
"""Trainium2 Bass kernel: Encoder_HieStackedCorr (UnCorrVmat_Detail, t_method='uncorr').

Math (per batch b):
    W1 = wn(U1_v, U1_g); W2 = wn(U2_v, U2_g)
    R = relu(V @ W1.T + b1)          [N, LR]
    L = relu(V @ W2.T + b2)          [N, LR]
    UnCorr = L @ R.T                 [N, N]
    d[n] = UnCorr[n, n] = sum_l L[n,l] R[n,l]
    dr = 1/sqrt(d + eps)
    A = 1 + I - dr dr^T * UnCorr
    v = mean_n (A @ V) = (1/N) * s @ V  where s[m] = N + 1 - dr[m] * (t . R[m,:]),
                                              t = sum_n dr[n] L[n,:]
    feat = v @ W_lin.T + b_lin ; out = batchnorm(feat)   (training-mode stats)

The N x N matrix is never materialized: mean-pooling commutes with the matmul,
collapsing the O(B N^2 (LR+D)) reference into O(B N D LR) work.

Sharding: data-parallel over batch, 4 batches per core on 8 cores.  Each core
computes v for its 4 batches; the tiny [32,256] linear + batchnorm epilogue
(needs cross-core batch stats) runs on host.
"""

import os
import sys

import numpy as np

for _p in ("/opt/trn_rl_repo", "/root/.axon_site/_ro/trn_rl_repo"):
    if os.path.isdir(_p) and _p not in sys.path:
        sys.path.insert(0, _p)
        break

import ml_dtypes  # noqa: E402
import concourse.bass as bass  # noqa: E402
import concourse.bacc as bacc  # noqa: E402
import concourse.mybir as mybir  # noqa: E402
import concourse.tile as tile  # noqa: E402
from concourse.bass_utils import run_bass_kernel_spmd  # noqa: E402


def _ensure_ntff_hook():
    """Shim the missing ``antenv.axon_hooks`` registry so trace=True works.

    The agent image's ``antenv`` lacks ``axon_hooks``; the ctypes NTFF driver
    in ``trn_agent_boot.trn_boot`` is present and the injected libaxon_pjrt.so
    exports the profile symbols, so wire them together here.
    """
    import types

    try:
        from antenv.axon_hooks import get_axon_ntff_profile_hook  # noqa: F401
        return
    except ImportError:
        pass
    try:
        from trn_agent_boot.trn_boot import _ntff_profile_via_ctypes
        hook = _ntff_profile_via_ctypes("/opt/axon/libaxon_pjrt.so")
    except Exception:
        hook = None
    mod = types.ModuleType("antenv.axon_hooks")
    mod._hook = hook
    mod.get_axon_ntff_profile_hook = lambda: mod._hook
    mod.set_axon_ntff_profile_hook = lambda h: setattr(mod, "_hook", h)
    sys.modules["antenv.axon_hooks"] = mod


_ensure_ntff_hook()

# Problem constants (hardcoded; see module docstring).
B, N, D, LR, EMB = 32, 2048, 256, 64, 256
NCORES = 8
B_LOC = B // NCORES          # 4 batches per core
ROWS = B_LOC * N             # 8192 rows per core
NT_B = N // 128              # 16 row-tiles per batch
NBLK = N // 512              # 4 512-row blocks per batch
EPS_DIAG = 1e-6
EPS_BN = 1e-5

F32 = mybir.dt.float32
BF16 = mybir.dt.bfloat16

# dt: dtype for V/Vt/W, the L/R activations and every matmul operand
# ("f32" = exact but 4 cyc/row on the PE, "bf16" = 1 cyc/row).
CONFIG = dict(dt="bf16", trace=False)

_CACHE = {}


def _build(cfg):
    DT = BF16 if cfg["dt"] == "bf16" else F32
    nc = bacc.Bacc("TRN2", target_bir_lowering=False, debug=False)

    v_d = nc.dram_tensor("v", [ROWS, D], DT, kind="ExternalInput").ap()
    vt_d = nc.dram_tensor("vt", [2, 128, ROWS], DT, kind="ExternalInput").ap()
    w1_d = nc.dram_tensor("w1t", [2, 128, LR], DT, kind="ExternalInput").ap()
    w2_d = nc.dram_tensor("w2t", [2, 128, LR], DT, kind="ExternalInput").ap()
    b1_d = nc.dram_tensor("b1", [LR, 1], F32, kind="ExternalInput").ap()
    b2_d = nc.dram_tensor("b2", [LR, 1], F32, kind="ExternalInput").ap()
    out_d = nc.dram_tensor("vmean", [1, B_LOC * D], F32, kind="ExternalOutput").ap()

    with tile.TileContext(nc) as tc:
        with (
            tc.tile_pool(name="const", bufs=1) as cpool,
            tc.tile_pool(name="vst", bufs=1) as vpool,
            tc.tile_pool(name="lrbuf", bufs=2) as lrpool,
            tc.tile_pool(name="blk", bufs=3) as bpool,
            tc.tile_pool(name="rows", bufs=2) as rpool,
            tc.tile_pool(name="ps_lr", bufs=2, space="PSUM") as ps_lr,
            tc.tile_pool(name="ps_d", bufs=1, space="PSUM") as ps_d,
            tc.tile_pool(name="ps_misc", bufs=1, space="PSUM") as ps_misc,
            tc.tile_pool(name="dram", bufs=2, space="DRAM") as dpool,
        ):
            # ---- constants / weights ----
            w1_sb = cpool.tile([128, 2 * LR], DT)
            w2_sb = cpool.tile([128, 2 * LR], DT)
            nc.sync.dma_start(
                w1_sb[:].rearrange("p (c l) -> p c l", c=2),
                w1_d.rearrange("c p l -> p c l"),
            )
            nc.sync.dma_start(
                w2_sb[:].rearrange("p (c l) -> p c l", c=2),
                w2_d.rearrange("c p l -> p c l"),
            )
            b1_sb = cpool.tile([LR, 1], F32)
            b2_sb = cpool.tile([LR, 1], F32)
            nc.sync.dma_start(b1_sb[:], b1_d[:])
            nc.sync.dma_start(b2_sb[:], b2_d[:])
            ones64 = cpool.tile([LR, 1], DT)
            nc.vector.memset(ones64[:], 1.0)
            ones_k1 = cpool.tile([1, LR], DT)
            nc.vector.memset(ones_k1[:], 1.0)
            eps_sb = cpool.tile([1, 1], F32)
            nc.vector.memset(eps_sb[:], EPS_DIAG)
            # Exp bias: dr' = exp(-0.5*ln(d+eps) - 0.5*ln N)
            expb_sb = cpool.tile([1, 1], F32)
            nc.vector.memset(expb_sb[:], -0.5 * float(np.log(N)))

            out_sb = cpool.tile([1, B_LOC * D], F32)

            # per-batch persistent tiles, double buffered across batches
            # DMA order: vt (needed first, per d-chunk so the first matmul only
            # waits on its own chunk), then the natural V (needed only at the
            # tail of each batch chain).
            v_t = {}
            vt_t = {}
            for b in range(B_LOC):
                v_t[b] = vpool.tile([128, NT_B * D], DT, tag="vnat", name=f"vnat{b}")
                vt_t[b] = vpool.tile([128, 2 * N], DT, tag="vt", name=f"vt{b}")
            for b in range(B_LOC):
                for c in range(2):
                    nc.sync.dma_start(
                        vt_t[b][:, c * N:(c + 1) * N],
                        vt_d[c, :, b * N:(b + 1) * N],
                    )
            for b in range(B_LOC):
                src = v_d.rearrange("(t p) d -> p t d", p=128)
                nc.sync.dma_start(
                    v_t[b][:].rearrange("p (t d) -> p t d", t=NT_B),
                    src[:, b * NT_B:(b + 1) * NT_B, :],
                )

            for b in range(B_LOC):
                L_sb = lrpool.tile([LR, N], DT, tag="L")
                R_sb = lrpool.tile([LR, N], DT, tag="R")
                ln_row = rpool.tile([1, N], F32, tag="ln")     # ln(d + eps)
                dr_row = rpool.tile([1, N], F32, tag="dr")     # rsqrt(d+eps)/sqrt(N)
                c_row = rpool.tile([1, N], F32, tag="c")       # c/N
                for blk in range(NBLK):
                    f0 = blk * 512
                    # L/R = V @ W.T in transposed layout [LR, n-block]
                    L_ps = ps_lr.tile([LR, 512], F32, tag="Lps")
                    R_ps = ps_lr.tile([LR, 512], F32, tag="Rps")
                    for c in range(2):
                        rhs = vt_t[b][:, c * N + f0:c * N + f0 + 512]
                        nc.tensor.matmul(
                            L_ps[:], w2_sb[:, c * LR:(c + 1) * LR], rhs,
                            start=(c == 0), stop=(c == 1),
                        )
                        nc.tensor.matmul(
                            R_ps[:], w1_sb[:, c * LR:(c + 1) * LR], rhs,
                            start=(c == 0), stop=(c == 1),
                        )
                    # relu(+bias): R on ACT, L on DVE (balance engines)
                    nc.scalar.activation(
                        R_sb[:, f0:f0 + 512], R_ps[:],
                        mybir.ActivationFunctionType.Relu, bias=b1_sb[:], scale=1.0,
                    )
                    nc.vector.tensor_scalar(
                        L_sb[:, f0:f0 + 512], L_ps[:], b2_sb[:], 0.0,
                        mybir.AluOpType.add, mybir.AluOpType.max,
                    )
                    # diag: d[n] = sum_l L[l,n]*R[l,n] -> [1,512] via ones-matmul
                    prod = bpool.tile([LR, 512], DT, tag="prod")
                    nc.vector.tensor_tensor(
                        prod[:], L_sb[:, f0:f0 + 512], R_sb[:, f0:f0 + 512],
                        mybir.AluOpType.mult,
                    )
                    d_ps = ps_d.tile([1, 512], F32, tag="dps")
                    nc.tensor.matmul(
                        d_ps[:], ones64[:], prod[:],
                        start=True, stop=True,
                    )
                    # dr' = rsqrt(d+eps)/sqrt(N) = exp(-0.5*ln(d+eps) - 0.5*ln N)
                    # (Rsqrt/Reciprocal are banned on ACT; DVE reciprocal is a
                    # 3.3us microcoded op -- Ln+Exp on ACT is 10x cheaper.)
                    nc.scalar.activation(
                        ln_row[:, f0:f0 + 512], d_ps[:],
                        mybir.ActivationFunctionType.Ln, bias=eps_sb[:], scale=1.0,
                    )
                    nc.scalar.activation(
                        dr_row[:, f0:f0 + 512], ln_row[:, f0:f0 + 512],
                        mybir.ActivationFunctionType.Exp,
                        bias=expb_sb[:], scale=-0.5,
                    )

                # t' = sum_n dr'[n] * L[n,:]   (chained fused multiply-reduce)
                dr_dt = dr_row
                if DT != F32:
                    # second Exp straight to bf16 (parallel, not a serial cast)
                    dr_dt = rpool.tile([1, N], DT, tag="dr_dt", name=f"drdt{b}")
                    for blk in range(NBLK):
                        f0 = blk * 512
                        nc.scalar.activation(
                            dr_dt[:, f0:f0 + 512], ln_row[:, f0:f0 + 512],
                            mybir.ActivationFunctionType.Exp,
                            bias=expb_sb[:], scale=-0.5,
                        )
                ldr = lrpool.tile([LR, N], DT, tag="ldr", name=f"ldr{b}")
                for blk in range(NBLK):
                    f0 = blk * 512
                    rep_ps = ps_misc.tile([LR, 512], F32, tag="rep")
                    nc.tensor.matmul(
                        rep_ps[:], ones_k1[:], dr_dt[:, f0:f0 + 512],
                        start=True, stop=True,
                    )
                    nc.vector.tensor_tensor(
                        ldr[:, f0:f0 + 512], L_sb[:, f0:f0 + 512], rep_ps[:],
                        mybir.AluOpType.mult,
                    )
                t_sb = bpool.tile([LR, 1], F32, tag="t", name=f"tacc{b}")
                nc.vector.tensor_reduce(
                    t_sb[:], ldr[:], mybir.AxisListType.X, mybir.AluOpType.add,
                )
                t_dt = t_sb
                if DT != F32:
                    t_dt = bpool.tile([LR, 1], DT, tag="t_dt", name=f"tdt{b}")
                    nc.scalar.activation(
                        t_dt[:], t_sb[:], mybir.ActivationFunctionType.Copy
                    )

                # u' = t' . R[m,:] -> [1,512] blocks; c' = u' * dr' = c/N
                # (the affine s = (N+1)/N - c' is folded into the tiny
                # post-scatter cast below)
                for blk in range(NBLK):
                    f0 = blk * 512
                    u_ps = ps_misc.tile([1, 512], F32, tag="ups")
                    nc.tensor.matmul(
                        u_ps[:], t_dt[:], R_sb[:, f0:f0 + 512],
                        start=True, stop=True,
                    )
                    nc.vector.tensor_tensor(
                        c_row[:, f0:f0 + 512], u_ps[:], dr_row[:, f0:f0 + 512],
                        mybir.AluOpType.mult,
                    )

                # scatter s to partitions: s_col[p, j] = s[j*128 + p].
                # A direct SBUF->SBUF rearrange is NOT usable: the source AP's
                # first dim is interpreted as physical partitions by the DMA
                # descriptor generator (HW reads partitions 1.. as garbage).
                # Bounce through DRAM, where APs are plain strided views.
                s_dram = dpool.tile([1, N], F32, tag="sdram", name=f"sdram{b}")
                nc.sync.dma_start(s_dram[:], c_row[:])
                s_col = bpool.tile([128, NT_B], F32, tag="scol")
                nc.sync.dma_start(
                    s_col[:], s_dram.rearrange("a (j p) -> (a p) j", p=128)
                )
                # s = (N+1)/N - c', fused with the DT cast on a [128,16] tile
                s_dt = bpool.tile([128, NT_B], DT, tag="scol_dt")
                nc.scalar.activation(
                    s_dt[:], s_col[:], mybir.ActivationFunctionType.Copy,
                    bias=float(N + 1) / N, scale=-1.0,
                )

                # v_mean = s^T @ V  (accumulate over the 16 row-tiles)
                v_ps = ps_misc.tile([1, D], F32, tag="vps")
                for j in range(NT_B):
                    nc.tensor.matmul(
                        v_ps[:], s_dt[:, j:j + 1],
                        v_t[b][:, j * D:(j + 1) * D],
                        start=(j == 0), stop=(j == NT_B - 1),
                    )
                nc.scalar.activation(
                    out_sb[:, b * D:(b + 1) * D], v_ps[:],
                    mybir.ActivationFunctionType.Copy,
                )

            nc.sync.dma_start(out_d[:], out_sb[:])

    nc.compile()
    return nc


def _host_prep(inputs, cfg):
    """Weight-norm, transposes, casts; returns per-core input maps + epilogue data."""
    np_dt = ml_dtypes.bfloat16 if cfg["dt"] == "bf16" else np.float32

    def wn(v, g):
        return v * (g / np.linalg.norm(v.astype(np.float64), axis=1)).astype(
            np.float32
        )[:, None]

    W1 = wn(np.asarray(inputs["U1_v"], np.float32), np.asarray(inputs["U1_g"], np.float32))
    W2 = wn(np.asarray(inputs["U2_v"], np.float32), np.asarray(inputs["U2_g"], np.float32))
    w1t = np.ascontiguousarray(W1.T).reshape(2, 128, LR).astype(np_dt)
    w2t = np.ascontiguousarray(W2.T).reshape(2, 128, LR).astype(np_dt)
    b1 = np.asarray(inputs["U1_b"], np.float32).reshape(LR, 1)
    b2 = np.asarray(inputs["U2_b"], np.float32).reshape(LR, 1)

    V = np.asarray(inputs["Vmat"], np.float32)  # [B, N, D]
    in_maps = []
    for k in range(NCORES):
        Vk = np.ascontiguousarray(V[k * B_LOC:(k + 1) * B_LOC].reshape(ROWS, D))
        vt = np.ascontiguousarray(Vk.T).reshape(2, 128, ROWS).astype(np_dt)
        in_maps.append({
            "v": Vk.astype(np_dt),
            "vt": vt,
            "w1t": w1t,
            "w2t": w2t,
            "b1": b1,
            "b2": b2,
        })
    return in_maps


def _epilogue(v_mean, inputs):
    """feat = v_mean @ W_lin.T + b_lin, then training-mode batchnorm."""
    W_lin = np.asarray(inputs["W_lin"], np.float32)
    b_lin = np.asarray(inputs["b_lin"], np.float32)
    gamma = np.asarray(inputs["gamma"], np.float32)
    beta = np.asarray(inputs["beta"], np.float32)
    feat = v_mean.astype(np.float32) @ W_lin.T + b_lin
    mu = feat.mean(axis=0)
    var = feat.var(axis=0)
    out = (feat - mu) / np.sqrt(var + EPS_BN) * gamma + beta
    return out.astype(np.float32)


def kernel(**inputs):
    cfg = dict(CONFIG)
    key = (cfg["dt"],)
    if key not in _CACHE:
        _CACHE[key] = _build(cfg)
    nc = _CACHE[key]
    in_maps = _host_prep(inputs, cfg)
    res = run_bass_kernel_spmd(
        nc, in_maps, core_ids=list(range(NCORES)), trace=cfg["trace"]
    )
    kernel.last_results = res
    v_mean = np.concatenate(
        [res.results[k]["vmean"].reshape(B_LOC, D) for k in range(NCORES)], axis=0
    )
    return _epilogue(v_mean, inputs)



# revision 15
# speedup vs baseline: 1.3576x; 1.2188x over previous
"""Trainium2 Bass kernel: Encoder_HieStackedCorr (UnCorrVmat_Detail, t_method='uncorr').

Math (per batch b):
    W1 = wn(U1_v, U1_g); W2 = wn(U2_v, U2_g)
    R = relu(V @ W1.T + b1)          [N, LR]
    L = relu(V @ W2.T + b2)          [N, LR]
    UnCorr = L @ R.T                 [N, N]
    d[n] = UnCorr[n, n] = sum_l L[n,l] R[n,l]
    dr = 1/sqrt(d + eps)
    A = 1 + I - dr dr^T * UnCorr
    v = mean_n (A @ V) = (1/N) * s @ V  where s[m] = N + 1 - dr[m] * (t . R[m,:]),
                                              t = sum_n dr[n] L[n,:]
    feat = v @ W_lin.T + b_lin ; out = batchnorm(feat)   (training-mode stats)

The N x N matrix is never materialized: mean-pooling commutes with the matmul,
collapsing the O(B N^2 (LR+D)) reference into O(B N D LR) work.

Sharding: data-parallel over batch, 4 batches per core on 8 cores.  Each core
computes v for its 4 batches; the tiny [32,256] linear + batchnorm epilogue
(needs cross-core batch stats) runs on host.
"""

import os
import sys

import numpy as np

for _p in ("/opt/trn_rl_repo", "/root/.axon_site/_ro/trn_rl_repo"):
    if os.path.isdir(_p) and _p not in sys.path:
        sys.path.insert(0, _p)
        break

import ml_dtypes  # noqa: E402
import concourse.bass as bass  # noqa: E402
import concourse.bacc as bacc  # noqa: E402
import concourse.bass_isa as bass_isa  # noqa: E402
import concourse.mybir as mybir  # noqa: E402
import concourse.tile as tile  # noqa: E402
from concourse.bass_utils import run_bass_kernel_spmd  # noqa: E402


def _ensure_ntff_hook():
    """Shim the missing ``antenv.axon_hooks`` registry so trace=True works.

    The agent image's ``antenv`` lacks ``axon_hooks``; the ctypes NTFF driver
    in ``trn_agent_boot.trn_boot`` is present and the injected libaxon_pjrt.so
    exports the profile symbols, so wire them together here.
    """
    import types

    try:
        from antenv.axon_hooks import get_axon_ntff_profile_hook  # noqa: F401
        return
    except ImportError:
        pass
    try:
        from trn_agent_boot.trn_boot import _ntff_profile_via_ctypes
        hook = _ntff_profile_via_ctypes("/opt/axon/libaxon_pjrt.so")
    except Exception:
        hook = None
    mod = types.ModuleType("antenv.axon_hooks")
    mod._hook = hook
    mod.get_axon_ntff_profile_hook = lambda: mod._hook
    mod.set_axon_ntff_profile_hook = lambda h: setattr(mod, "_hook", h)
    sys.modules["antenv.axon_hooks"] = mod


_ensure_ntff_hook()

# Problem constants (hardcoded; see module docstring).
B, N, D, LR, EMB = 32, 2048, 256, 64, 256
NCORES = 8
B_LOC = B // NCORES          # 4 batches per core
ROWS = B_LOC * N             # 8192 rows per core
NT_B = N // 128              # 16 row-tiles per batch
NBLK = N // 512              # 4 512-row blocks per batch
EPS_DIAG = 1e-6
EPS_BN = 1e-5

F32 = mybir.dt.float32
BF16 = mybir.dt.bfloat16

# dt: dtype for V/Vt/W, the L/R activations and every matmul operand
# ("f32" = exact but 4 cyc/row on the PE, "bf16" = 1 cyc/row).
CONFIG = dict(dt="bf16", trace=False)

_CACHE = {}


def _build(cfg):
    DT = BF16 if cfg["dt"] == "bf16" else F32
    nc = bacc.Bacc("TRN2", target_bir_lowering=False, debug=False)

    v_d = nc.dram_tensor("v", [ROWS, D], DT, kind="ExternalInput").ap()
    vt_d = nc.dram_tensor("vt", [2, 128, ROWS], DT, kind="ExternalInput").ap()
    w1_d = nc.dram_tensor("w1t", [2, 128, LR], DT, kind="ExternalInput").ap()
    w2_d = nc.dram_tensor("w2t", [2, 128, LR], DT, kind="ExternalInput").ap()
    b1_d = nc.dram_tensor("b1", [LR, 1], F32, kind="ExternalInput").ap()
    b2_d = nc.dram_tensor("b2", [LR, 1], F32, kind="ExternalInput").ap()
    out_d = nc.dram_tensor("vmean", [1, B_LOC * D], F32, kind="ExternalOutput").ap()

    with tile.TileContext(nc) as tc:
        with (
            tc.tile_pool(name="const", bufs=1) as cpool,
            tc.tile_pool(name="vst", bufs=1) as vpool,
            tc.tile_pool(name="lrbuf", bufs=2) as lrpool,
            tc.tile_pool(name="blk", bufs=3) as bpool,
            tc.tile_pool(name="rows", bufs=2) as rpool,
            tc.tile_pool(name="ps_lr", bufs=2, space="PSUM") as ps_lr,
            tc.tile_pool(name="ps_d", bufs=1, space="PSUM") as ps_d,
            tc.tile_pool(name="ps_misc", bufs=1, space="PSUM") as ps_misc,
            tc.tile_pool(name="dram", bufs=2, space="DRAM") as dpool,
        ):
            # ---- constants / weights ----
            w1_sb = cpool.tile([128, 2 * LR], DT)
            w2_sb = cpool.tile([128, 2 * LR], DT)
            nc.sync.dma_start(
                w1_sb[:].rearrange("p (c l) -> p c l", c=2),
                w1_d.rearrange("c p l -> p c l"),
            )
            nc.sync.dma_start(
                w2_sb[:].rearrange("p (c l) -> p c l", c=2),
                w2_d.rearrange("c p l -> p c l"),
            )
            b1_sb = cpool.tile([LR, 1], F32)
            b2_sb = cpool.tile([LR, 1], F32)
            nc.sync.dma_start(b1_sb[:], b1_d[:])
            nc.sync.dma_start(b2_sb[:], b2_d[:])
            ones64 = cpool.tile([LR, 1], DT)
            nc.vector.memset(ones64[:], 1.0)
            ones_k1 = cpool.tile([1, LR], DT)
            nc.vector.memset(ones_k1[:], 1.0)
            eps_sb = cpool.tile([1, 1], F32)
            nc.vector.memset(eps_sb[:], EPS_DIAG)
            # Exp bias: dr' = exp(-0.5*ln(d+eps) - 0.5*ln N) = rsqrt(d+eps)/sqrt(N)
            expb_sb = cpool.tile([1, 1], F32)
            nc.vector.memset(expb_sb[:], -0.5 * float(np.log(N)))

            out_sb = cpool.tile([1, B_LOC * D], F32)

            # per-batch persistent tiles, double buffered across batches
            # DMA order: vt (needed first, per d-chunk so the first matmul only
            # waits on its own chunk), then the natural V (needed only at the
            # tail of each batch chain).
            v_t = {}
            vt_t = {}
            for b in range(B_LOC):
                v_t[b] = vpool.tile([128, NT_B * D], DT, tag="vnat", name=f"vnat{b}")
                vt_t[b] = vpool.tile([128, 2 * N], DT, tag="vt", name=f"vt{b}")
            for b in range(B_LOC):
                for c in range(2):
                    nc.sync.dma_start(
                        vt_t[b][:, c * N:(c + 1) * N],
                        vt_d[c, :, b * N:(b + 1) * N],
                    )
            for b in range(B_LOC):
                src = v_d.rearrange("(t p) d -> p t d", p=128)
                nc.sync.dma_start(
                    v_t[b][:].rearrange("p (t d) -> p t d", t=NT_B),
                    src[:, b * NT_B:(b + 1) * NT_B, :],
                )

            for b in range(B_LOC):
                L_sb = lrpool.tile([LR, N], DT, tag="L")
                R_sb = lrpool.tile([LR, N], DT, tag="R")
                ln_row = rpool.tile([1, N], F32, tag="ln")   # ln(d + eps)
                dr_row = rpool.tile([1, N], F32, tag="dr")   # rsqrt(d+eps)/sqrt(N)
                dr_dt = dr_row
                if DT != F32:
                    dr_dt = rpool.tile([1, N], DT, tag="dr_dt")
                c_row = rpool.tile([1, N], F32, tag="c")     # c/N
                for blk in range(NBLK):
                    f0 = blk * 512
                    # L/R = V @ W.T in transposed layout [LR, n-block]
                    L_ps = ps_lr.tile([LR, 512], F32, tag="Lps")
                    R_ps = ps_lr.tile([LR, 512], F32, tag="Rps")
                    for c in range(2):
                        rhs = vt_t[b][:, c * N + f0:c * N + f0 + 512]
                        nc.tensor.matmul(
                            L_ps[:], w2_sb[:, c * LR:(c + 1) * LR], rhs,
                            start=(c == 0), stop=(c == 1),
                        )
                        nc.tensor.matmul(
                            R_ps[:], w1_sb[:, c * LR:(c + 1) * LR], rhs,
                            start=(c == 0), stop=(c == 1),
                        )
                    # relu(+bias): R on ACT, L on DVE (balance engines)
                    nc.scalar.activation(
                        R_sb[:, f0:f0 + 512], R_ps[:],
                        mybir.ActivationFunctionType.Relu, bias=b1_sb[:], scale=1.0,
                    )
                    nc.vector.tensor_scalar(
                        L_sb[:, f0:f0 + 512], L_ps[:], b2_sb[:], 0.0,
                        mybir.AluOpType.add, mybir.AluOpType.max,
                    )
                    # diag: d[n] = sum_l L[l,n]*R[l,n] -> [1,512] via ones-matmul
                    prod = bpool.tile([LR, 512], DT, tag="prod")
                    nc.vector.tensor_tensor(
                        prod[:], L_sb[:, f0:f0 + 512], R_sb[:, f0:f0 + 512],
                        mybir.AluOpType.mult,
                    )
                    d_ps = ps_d.tile([1, 512], F32, tag="dps")
                    nc.tensor.matmul(
                        d_ps[:], ones64[:], prod[:],
                        start=True, stop=True,
                    )
                    # Rsqrt/Reciprocal are banned on ACT and DVE reciprocal is
                    # a 3.3us microcoded op: use exp(-0.5*ln(d+eps) - 0.5 ln N).
                    # Ln per block, the two Exps once per batch: Relu/Copy
                    # between Ln calls don't touch the ACT function table, so
                    # this order costs only 2 ACT_TABLE_LOADs per batch.
                    nc.scalar.activation(
                        ln_row[:, f0:f0 + 512], d_ps[:],
                        mybir.ActivationFunctionType.Ln, bias=eps_sb[:], scale=1.0,
                    )
                nc.scalar.activation(
                    dr_row[:], ln_row[:],
                    mybir.ActivationFunctionType.Exp, bias=expb_sb[:], scale=-0.5,
                )
                if DT != F32:
                    nc.scalar.activation(
                        dr_dt[:], ln_row[:],
                        mybir.ActivationFunctionType.Exp, bias=expb_sb[:], scale=-0.5,
                    )

                # t'[l] = sum_n dr'[n]*L[l,n]: broadcast dr' to LR partitions
                # via ones-matmul, multiply, then one reduce over [LR, N].
                # (tensor_tensor_reduce and the gpsimd partition ops crash the
                # HW runtime here -- no HIPI ucode on bedrock; see ttr_test.py.)
                ldr = lrpool.tile([LR, N], DT, tag="ldr")
                for blk in range(NBLK):
                    f0 = blk * 512
                    rep_ps = ps_misc.tile([LR, 512], F32, tag="rep")
                    nc.tensor.matmul(
                        rep_ps[:], ones_k1[:], dr_dt[:, f0:f0 + 512],
                        start=True, stop=True,
                    )
                    nc.vector.tensor_tensor(
                        ldr[:, f0:f0 + 512], L_sb[:, f0:f0 + 512], rep_ps[:],
                        mybir.AluOpType.mult,
                    )
                t_sb = bpool.tile([LR, 1], F32, tag="t", name=f"tacc{b}")
                nc.vector.tensor_reduce(
                    t_sb[:], ldr[:], mybir.AxisListType.X, mybir.AluOpType.add,
                )
                t_dt = t_sb
                if DT != F32:
                    t_dt = bpool.tile([LR, 1], DT, tag="t_dt")
                    nc.scalar.activation(
                        t_dt[:], t_sb[:], mybir.ActivationFunctionType.Copy
                    )

                # u' = t' . R[m,:] -> [1,512] blocks; c' = u' * dr' = c/N
                # (the affine s = (N+1)/N - c' is folded into the tiny
                # post-scatter cast below)
                for blk in range(NBLK):
                    f0 = blk * 512
                    u_ps = ps_misc.tile([1, 512], F32, tag="ups")
                    nc.tensor.matmul(
                        u_ps[:], t_dt[:], R_sb[:, f0:f0 + 512],
                        start=True, stop=True,
                    )
                    nc.vector.tensor_tensor(
                        c_row[:, f0:f0 + 512], u_ps[:], dr_row[:, f0:f0 + 512],
                        mybir.AluOpType.mult,
                    )

                # scatter s to partitions: s_col[p, j] = s[j*128 + p].
                # A direct SBUF->SBUF rearrange is NOT usable: the source AP's
                # first dim is interpreted as physical partitions by the DMA
                # descriptor generator (HW reads partitions 1.. as garbage).
                # Bounce through DRAM, where APs are plain strided views.
                s_dram = dpool.tile([1, N], F32, tag="sdram", name=f"sdram{b}")
                nc.sync.dma_start(s_dram[:], c_row[:])
                s_col = bpool.tile([128, NT_B], F32, tag="scol")
                nc.sync.dma_start(
                    s_col[:], s_dram.rearrange("a (j p) -> (a p) j", p=128)
                )
                # s = (N+1)/N - c', fused with the DT cast on a [128,16] tile
                s_dt = bpool.tile([128, NT_B], DT, tag="scol_dt")
                nc.scalar.activation(
                    s_dt[:], s_col[:], mybir.ActivationFunctionType.Copy,
                    bias=float(N + 1) / N, scale=-1.0,
                )

                # v_mean = s^T @ V  (accumulate over the 16 row-tiles)
                v_ps = ps_misc.tile([1, D], F32, tag="vps")
                for j in range(NT_B):
                    nc.tensor.matmul(
                        v_ps[:], s_dt[:, j:j + 1],
                        v_t[b][:, j * D:(j + 1) * D],
                        start=(j == 0), stop=(j == NT_B - 1),
                    )
                nc.scalar.activation(
                    out_sb[:, b * D:(b + 1) * D], v_ps[:],
                    mybir.ActivationFunctionType.Copy,
                )

            nc.sync.dma_start(out_d[:], out_sb[:])

    nc.compile()
    return nc


def _host_prep(inputs, cfg):
    """Weight-norm, transposes, casts; returns per-core input maps + epilogue data."""
    np_dt = ml_dtypes.bfloat16 if cfg["dt"] == "bf16" else np.float32

    def wn(v, g):
        return v * (g / np.linalg.norm(v.astype(np.float64), axis=1)).astype(
            np.float32
        )[:, None]

    W1 = wn(np.asarray(inputs["U1_v"], np.float32), np.asarray(inputs["U1_g"], np.float32))
    W2 = wn(np.asarray(inputs["U2_v"], np.float32), np.asarray(inputs["U2_g"], np.float32))
    w1t = np.ascontiguousarray(W1.T).reshape(2, 128, LR).astype(np_dt)
    w2t = np.ascontiguousarray(W2.T).reshape(2, 128, LR).astype(np_dt)
    b1 = np.asarray(inputs["U1_b"], np.float32).reshape(LR, 1)
    b2 = np.asarray(inputs["U2_b"], np.float32).reshape(LR, 1)

    V = np.asarray(inputs["Vmat"], np.float32)  # [B, N, D]
    in_maps = []
    for k in range(NCORES):
        Vk = np.ascontiguousarray(V[k * B_LOC:(k + 1) * B_LOC].reshape(ROWS, D))
        vt = np.ascontiguousarray(Vk.T).reshape(2, 128, ROWS).astype(np_dt)
        in_maps.append({
            "v": Vk.astype(np_dt),
            "vt": vt,
            "w1t": w1t,
            "w2t": w2t,
            "b1": b1,
            "b2": b2,
        })
    return in_maps


def _epilogue(v_mean, inputs):
    """feat = v_mean @ W_lin.T + b_lin, then training-mode batchnorm."""
    W_lin = np.asarray(inputs["W_lin"], np.float32)
    b_lin = np.asarray(inputs["b_lin"], np.float32)
    gamma = np.asarray(inputs["gamma"], np.float32)
    beta = np.asarray(inputs["beta"], np.float32)
    feat = v_mean.astype(np.float32) @ W_lin.T + b_lin
    mu = feat.mean(axis=0)
    var = feat.var(axis=0)
    out = (feat - mu) / np.sqrt(var + EPS_BN) * gamma + beta
    return out.astype(np.float32)


def kernel(**inputs):
    cfg = dict(CONFIG)
    key = (cfg["dt"],)
    if key not in _CACHE:
        _CACHE[key] = _build(cfg)
    nc = _CACHE[key]
    in_maps = _host_prep(inputs, cfg)
    res = run_bass_kernel_spmd(
        nc, in_maps, core_ids=list(range(NCORES)), trace=cfg["trace"]
    )
    kernel.last_results = res
    v_mean = np.concatenate(
        [res.results[k]["vmean"].reshape(B_LOC, D) for k in range(NCORES)], axis=0
    )
    return _epilogue(v_mean, inputs)



# revision 18
# speedup vs baseline: 1.5141x; 1.1153x over previous
"""Trainium2 Bass kernel: Encoder_HieStackedCorr (UnCorrVmat_Detail, t_method='uncorr').

Math (per batch b):
    W1 = wn(U1_v, U1_g); W2 = wn(U2_v, U2_g)
    R = relu(V @ W1.T + b1)          [N, LR]
    L = relu(V @ W2.T + b2)          [N, LR]
    UnCorr = L @ R.T                 [N, N]
    d[n] = UnCorr[n, n] = sum_l L[n,l] R[n,l]
    dr = 1/sqrt(d + eps)
    A = 1 + I - dr dr^T * UnCorr
    v = mean_n (A @ V) = (1/N) * s @ V  where s[m] = N + 1 - dr[m] * (t . R[m,:]),
                                              t = sum_n dr[n] L[n,:]
    feat = v @ W_lin.T + b_lin ; out = batchnorm(feat)   (training-mode stats)

The N x N matrix is never materialized: mean-pooling commutes with the matmul,
collapsing the O(B N^2 (LR+D)) reference into O(B N D LR) work.

Sharding: data-parallel over batch, 4 batches per core on 8 cores.  Each core
computes v for its 4 batches; the tiny [32,256] linear + batchnorm epilogue
(needs cross-core batch stats) runs on host.
"""

import os
import sys

import numpy as np

for _p in ("/opt/trn_rl_repo", "/root/.axon_site/_ro/trn_rl_repo"):
    if os.path.isdir(_p) and _p not in sys.path:
        sys.path.insert(0, _p)
        break

import ml_dtypes  # noqa: E402
import concourse.bass as bass  # noqa: E402
import concourse.bacc as bacc  # noqa: E402
import concourse.bass_isa as bass_isa  # noqa: E402
import concourse.mybir as mybir  # noqa: E402
import concourse.tile as tile  # noqa: E402
from concourse.bass_utils import run_bass_kernel_spmd  # noqa: E402


def _ensure_ntff_hook():
    """Shim the missing ``antenv.axon_hooks`` registry so trace=True works.

    The agent image's ``antenv`` lacks ``axon_hooks``; the ctypes NTFF driver
    in ``trn_agent_boot.trn_boot`` is present and the injected libaxon_pjrt.so
    exports the profile symbols, so wire them together here.
    """
    import types

    try:
        from antenv.axon_hooks import get_axon_ntff_profile_hook  # noqa: F401
        return
    except ImportError:
        pass
    try:
        from trn_agent_boot.trn_boot import _ntff_profile_via_ctypes
        hook = _ntff_profile_via_ctypes("/opt/axon/libaxon_pjrt.so")
    except Exception:
        hook = None
    mod = types.ModuleType("antenv.axon_hooks")
    mod._hook = hook
    mod.get_axon_ntff_profile_hook = lambda: mod._hook
    mod.set_axon_ntff_profile_hook = lambda h: setattr(mod, "_hook", h)
    sys.modules["antenv.axon_hooks"] = mod


_ensure_ntff_hook()

# Problem constants (hardcoded; see module docstring).
B, N, D, LR, EMB = 32, 2048, 256, 64, 256
NCORES = 8
B_LOC = B // NCORES          # 4 batches per core
ROWS = B_LOC * N             # 8192 rows per core
NT_B = N // 128              # 16 row-tiles per batch
NBLK = N // 512              # 4 512-row blocks per batch
EPS_DIAG = 1e-6
EPS_BN = 1e-5

F32 = mybir.dt.float32
BF16 = mybir.dt.bfloat16

# dt: dtype for V/Vt/W, the L/R activations and every matmul operand
# ("f32" = exact but 4 cyc/row on the PE, "bf16" = 1 cyc/row).
CONFIG = dict(dt="bf16", trace=False)

_CACHE = {}


def _build(cfg):
    DT = BF16 if cfg["dt"] == "bf16" else F32
    nc = bacc.Bacc("TRN2", target_bir_lowering=False, debug=False)

    v_d = nc.dram_tensor("v", [ROWS, D], DT, kind="ExternalInput").ap()
    vt_d = nc.dram_tensor("vt", [2, 128, ROWS], DT, kind="ExternalInput").ap()
    w12_d = nc.dram_tensor("w12t", [2, 128, 2 * LR], DT, kind="ExternalInput").ap()
    b12_d = nc.dram_tensor("b12", [2 * LR, 1], F32, kind="ExternalInput").ap()
    out_d = nc.dram_tensor("vmean", [1, B_LOC * D], F32, kind="ExternalOutput").ap()

    with tile.TileContext(nc) as tc:
        with (
            tc.tile_pool(name="const", bufs=1) as cpool,
            tc.tile_pool(name="vst", bufs=1) as vpool,
            tc.tile_pool(name="lrbuf", bufs=4) as lrpool,
            tc.tile_pool(name="blk", bufs=6) as bpool,
            tc.tile_pool(name="rows", bufs=3) as rpool,
            tc.tile_pool(name="ps_lr", bufs=2, space="PSUM") as ps_lr,
            tc.tile_pool(name="ps_d", bufs=2, space="PSUM") as ps_d,
            tc.tile_pool(name="ps_misc", bufs=1, space="PSUM") as ps_misc,
            tc.tile_pool(name="dram", bufs=4, space="DRAM") as dpool,
        ):
            # ---- constants / weights ----
            # w12 packs [W2 | W1] so one 128-wide matmul emits L rows 0:64 and
            # R rows 64:128 in a single PE pass; b12 = [b2; b1] to match.
            w12_sb = cpool.tile([128, 2 * 2 * LR], DT)
            nc.sync.dma_start(
                w12_sb[:].rearrange("p (c l) -> p c l", c=2),
                w12_d.rearrange("c p l -> p c l"),
            )
            b12_sb = cpool.tile([2 * LR, 1], F32)
            nc.sync.dma_start(b12_sb[:], b12_d[:])
            ones64 = cpool.tile([LR, 1], DT)
            nc.vector.memset(ones64[:], 1.0)
            ones_k1 = cpool.tile([1, LR], DT)
            nc.vector.memset(ones_k1[:], 1.0)
            eps_sb = cpool.tile([1, 1], F32)
            nc.vector.memset(eps_sb[:], EPS_DIAG)
            # Exp bias: dr' = exp(-0.5*ln(d+eps) - 0.5*ln N) = rsqrt(d+eps)/sqrt(N)
            expb_sb = cpool.tile([1, 1], F32)
            nc.vector.memset(expb_sb[:], -0.5 * float(np.log(N)))

            out_sb = cpool.tile([1, B_LOC * D], F32)

            # per-batch persistent tiles, double buffered across batches
            # DMA order: vt (needed first, per d-chunk so the first matmul only
            # waits on its own chunk), then the natural V (needed only at the
            # tail of each batch chain).
            v_t = {}
            vt_t = {}
            for b in range(B_LOC):
                v_t[b] = vpool.tile([128, NT_B * D], DT, tag="vnat", name=f"vnat{b}")
                vt_t[b] = vpool.tile([128, 2 * N], DT, tag="vt", name=f"vt{b}")
            for b in range(B_LOC):
                for c in range(2):
                    nc.sync.dma_start(
                        vt_t[b][:, c * N:(c + 1) * N],
                        vt_d[c, :, b * N:(b + 1) * N],
                    )
            # row n of batch b lives at (partition, tile) = (n // 16, n % 16):
            # per partition the DMA reads 16*256 contiguous elements (one 8KB
            # descriptor) instead of 16 strided 512B ones; the s-scatter gather
            # below then reads 16 consecutive floats (64B) per partition.
            src = v_d.rearrange("(b p t) d -> b p t d", p=128, t=NT_B)
            for b in range(B_LOC):
                nc.sync.dma_start(
                    v_t[b][:].rearrange("p (t d) -> p t d", t=NT_B),
                    src[b],
                )

            for b in range(B_LOC):
                L_sb = lrpool.tile([LR, N], DT, tag="L")
                R_sb = lrpool.tile([LR, N], DT, tag="R")
                ln_row = rpool.tile([1, N], F32, tag="ln")   # ln(d + eps)
                dr_dt = rpool.tile([1, N], DT, tag="dr")     # rsqrt(d+eps)/sqrt(N)
                c_row = rpool.tile([1, N], F32, tag="c")     # c/N
                tpart = bpool.tile([LR, NBLK], F32, tag="tpart")
                for blk in range(NBLK):
                    f0 = blk * 512
                    # [L; R] = V @ [W2|W1].T in one 128-wide PE pass per chunk
                    LR_ps = ps_lr.tile([128, 512], F32, tag="LRps")
                    for c in range(2):
                        rhs = vt_t[b][:, c * N + f0:c * N + f0 + 512]
                        nc.tensor.matmul(
                            LR_ps[:], w12_sb[:, c * 2 * LR:(c + 1) * 2 * LR], rhs,
                            start=(c == 0), stop=(c == 1),
                        )
                    # split: R (psum rows 64:128) on ACT, L (rows 0:64) on DVE
                    # (both engines may read PSUM at these base partitions, but
                    # DVE may not mix base 0 and 64 operands in one op)
                    nc.scalar.activation(
                        R_sb[:, f0:f0 + 512], LR_ps[LR:2 * LR, :],
                        mybir.ActivationFunctionType.Relu,
                        bias=b12_sb[LR:2 * LR], scale=1.0,
                    )
                    nc.vector.tensor_scalar(
                        L_sb[:, f0:f0 + 512], LR_ps[0:LR, :], b12_sb[0:LR], 0.0,
                        mybir.AluOpType.add, mybir.AluOpType.max,
                    )
                    # diag: d[n] = sum_l L[l,n]*R[l,n] -> [1,512] via ones-matmul
                    prod = bpool.tile([LR, 512], DT, tag="prod")
                    nc.vector.tensor_tensor(
                        prod[:], L_sb[:, f0:f0 + 512], R_sb[:, f0:f0 + 512],
                        mybir.AluOpType.mult,
                    )
                    d_ps = ps_d.tile([1, 512], F32, tag="dps")
                    nc.tensor.matmul(
                        d_ps[:], ones64[:], prod[:],
                        start=True, stop=True,
                    )
                    # Rsqrt/Reciprocal are banned on ACT and DVE reciprocal is
                    # a 3.3us microcoded op: use exp(-0.5*ln(d+eps) - 0.5 ln N).
                    # Ln per block, one Exp per batch: Relu/Copy between Ln
                    # calls don't touch the ACT function table, so this order
                    # costs only 2 ACT_TABLE_LOADs per batch.
                    nc.scalar.activation(
                        ln_row[:, f0:f0 + 512], d_ps[:],
                        mybir.ActivationFunctionType.Ln, bias=eps_sb[:], scale=1.0,
                    )
                nc.scalar.activation(
                    dr_dt[:], ln_row[:],
                    mybir.ActivationFunctionType.Exp, bias=expb_sb[:], scale=-0.5,
                )

                # t'[l] = sum_n dr'[n]*L[l,n]: broadcast dr' to LR partitions
                # via ones-matmul, multiply, per-block partial reduces (so the
                # final reduce is tiny), then combine.
                ldr = lrpool.tile([LR, N], DT, tag="ldr")
                for blk in range(NBLK):
                    f0 = blk * 512
                    rep_ps = ps_misc.tile([LR, 512], F32, tag="rep")
                    nc.tensor.matmul(
                        rep_ps[:], ones_k1[:], dr_dt[:, f0:f0 + 512],
                        start=True, stop=True,
                    )
                    nc.vector.tensor_tensor(
                        ldr[:, f0:f0 + 512], L_sb[:, f0:f0 + 512], rep_ps[:],
                        mybir.AluOpType.mult,
                    )
                    nc.vector.tensor_reduce(
                        tpart[:, blk:blk + 1], ldr[:, f0:f0 + 512],
                        mybir.AxisListType.X, mybir.AluOpType.add,
                    )
                t_sb = bpool.tile([LR, 1], F32, tag="t")
                nc.vector.tensor_reduce(
                    t_sb[:], tpart[:], mybir.AxisListType.X, mybir.AluOpType.add,
                )
                t_dt = t_sb
                if DT != F32:
                    t_dt = bpool.tile([LR, 1], DT, tag="t_dt")
                    nc.scalar.activation(
                        t_dt[:], t_sb[:], mybir.ActivationFunctionType.Copy
                    )

                # u' = t' . R[m,:] -> [1,512] blocks; c' = u' * dr' = c/N
                # (the affine s = (N+1)/N - c' is folded into the tiny
                # post-scatter cast below; ups ping-pongs across two PSUM tags)
                for blk in range(NBLK):
                    f0 = blk * 512
                    u_ps = ps_misc.tile([1, 512], F32, tag=f"ups{blk % 2}")
                    nc.tensor.matmul(
                        u_ps[:], t_dt[:], R_sb[:, f0:f0 + 512],
                        start=True, stop=True,
                    )
                    nc.vector.tensor_tensor(
                        c_row[:, f0:f0 + 512], u_ps[:], dr_dt[:, f0:f0 + 512],
                        mybir.AluOpType.mult,
                    )

                # scatter s to partitions: s_col[p, j] = s[p*16 + j].
                # A direct SBUF->SBUF rearrange is NOT usable: the source AP's
                # first dim is interpreted as physical partitions by the DMA
                # descriptor generator (HW reads partitions 1.. as garbage).
                # Bounce through DRAM, where APs are plain strided views.
                s_dram = dpool.tile([1, N], F32, tag="sdram", name=f"sdram{b}")
                nc.sync.dma_start(s_dram[:], c_row[:])
                s_col = bpool.tile([128, NT_B], F32, tag="scol")
                nc.sync.dma_start(
                    s_col[:], s_dram.rearrange("a (p j) -> (a p) j", p=128)
                )
                # s = (N+1)/N - c', fused with the DT cast on a [128,16] tile
                s_dt = bpool.tile([128, NT_B], DT, tag="scol_dt")
                nc.scalar.activation(
                    s_dt[:], s_col[:], mybir.ActivationFunctionType.Copy,
                    bias=float(N + 1) / N, scale=-1.0,
                )

                # v_mean = s^T @ V  (accumulate over the 16 row-tiles)
                v_ps = ps_misc.tile([1, D], F32, tag="vps")
                for j in range(NT_B):
                    nc.tensor.matmul(
                        v_ps[:], s_dt[:, j:j + 1],
                        v_t[b][:, j * D:(j + 1) * D],
                        start=(j == 0), stop=(j == NT_B - 1),
                    )
                nc.scalar.activation(
                    out_sb[:, b * D:(b + 1) * D], v_ps[:],
                    mybir.ActivationFunctionType.Copy,
                )
                nc.sync.dma_start(
                    out_d[:, b * D:(b + 1) * D], out_sb[:, b * D:(b + 1) * D]
                )

    nc.compile()
    return nc


def _host_prep(inputs, cfg):
    """Weight-norm, transposes, casts; returns per-core input maps + epilogue data."""
    np_dt = ml_dtypes.bfloat16 if cfg["dt"] == "bf16" else np.float32

    def wn(v, g):
        return v * (g / np.linalg.norm(v.astype(np.float64), axis=1)).astype(
            np.float32
        )[:, None]

    W1 = wn(np.asarray(inputs["U1_v"], np.float32), np.asarray(inputs["U1_g"], np.float32))
    W2 = wn(np.asarray(inputs["U2_v"], np.float32), np.asarray(inputs["U2_g"], np.float32))
    w1t = np.ascontiguousarray(W1.T).reshape(2, 128, LR).astype(np_dt)
    w2t = np.ascontiguousarray(W2.T).reshape(2, 128, LR).astype(np_dt)
    # combined lhsT [W2 | W1] and stacked bias [b2; b1] (L rows 0:64, R 64:128)
    w12t = np.ascontiguousarray(np.concatenate([w2t, w1t], axis=2))
    b1 = np.asarray(inputs["U1_b"], np.float32).reshape(LR, 1)
    b2 = np.asarray(inputs["U2_b"], np.float32).reshape(LR, 1)
    b12 = np.ascontiguousarray(np.concatenate([b2, b1], axis=0))

    V = np.asarray(inputs["Vmat"], np.float32)  # [B, N, D]
    in_maps = []
    for k in range(NCORES):
        Vk = np.ascontiguousarray(V[k * B_LOC:(k + 1) * B_LOC].reshape(ROWS, D))
        vt = np.ascontiguousarray(Vk.T).reshape(2, 128, ROWS).astype(np_dt)
        in_maps.append({
            "v": Vk.astype(np_dt),
            "vt": vt,
            "w12t": w12t,
            "b12": b12,
        })
    return in_maps


def _epilogue(v_mean, inputs):
    """feat = v_mean @ W_lin.T + b_lin, then training-mode batchnorm."""
    W_lin = np.asarray(inputs["W_lin"], np.float32)
    b_lin = np.asarray(inputs["b_lin"], np.float32)
    gamma = np.asarray(inputs["gamma"], np.float32)
    beta = np.asarray(inputs["beta"], np.float32)
    feat = v_mean.astype(np.float32) @ W_lin.T + b_lin
    mu = feat.mean(axis=0)
    var = feat.var(axis=0)
    out = (feat - mu) / np.sqrt(var + EPS_BN) * gamma + beta
    return out.astype(np.float32)


def kernel(**inputs):
    cfg = dict(CONFIG)
    key = (cfg["dt"],)
    if key not in _CACHE:
        _CACHE[key] = _build(cfg)
    nc = _CACHE[key]
    in_maps = _host_prep(inputs, cfg)
    res = run_bass_kernel_spmd(
        nc, in_maps, core_ids=list(range(NCORES)), trace=cfg["trace"]
    )
    kernel.last_results = res
    v_mean = np.concatenate(
        [res.results[k]["vmean"].reshape(B_LOC, D) for k in range(NCORES)], axis=0
    )
    return _epilogue(v_mean, inputs)

